# revision 10
# baseline (speedup 1.0000x reference)
"""Trainium2 Bass kernel for nn_MultiHeadAttention_4389456577178.

Module: x -> conv1x1 (qkv1) -> conv3x3 SAME (qkv2) -> split q,k,v
        -> l2norm(q), l2norm(k) over W -> per-channel spatial attention
        (attn = softmax(scale_c * Q K^T) @ V over the H dimension,
        contracting W) -> conv1x1 (proj).

Sharding (8 cores): core = 2*batch + half. Each core handles one batch
image and one half (64) of the attention channels. The qkv2 conv output
channels are split accordingly (192 per core: q/k/v halves); qkv1 output
(the conv2 input) is recomputed fully on both cores of a pair (cheap).
The proj conv contracts all 128 channels, so each core computes a
partial proj output from its 64 channels and the host adds the pair.

On-chip dataflow per core (all spatial = 128x128):
  conv1+conv2 fused over 16 row-blocks of 8 output rows, fp32r matmuls
  (TF32-class precision at full PE speed); q,k l2-normalized (+ q scaled
  by `scale`) in a batched pass; attention per channel with PE-transposes
  and exact fp32 matmuls; proj as fp32 matmuls; host adds core pairs.
"""

import threading

import numpy as np

import concourse.bass as bass
import concourse.tile as tile
from concourse import bacc, mybir
from concourse.bass_utils import run_bass_kernel_spmd
from concourse.masks import make_identity

F32 = mybir.dt.float32
F32R = mybir.dt.float32r
AX = mybir.AxisListType
AF = mybir.ActivationFunctionType

B, C, H, W = 4, 128, 128, 128
HW = H * W
NCORES = 8
NBLK = 16          # conv row blocks
BLKR = 8           # output rows per block
TROWS = BLKR + 2   # t rows held per block (halo)


def _trace_kernel(tc, x_d, w1_d, w2_d, pw_d, qsc_d, y_d):
    nc = tc.nc
    import contextlib

    with contextlib.ExitStack() as ctx:
        # ---------------- constants (whole-kernel lifetime) ----------------
        const = ctx.enter_context(tc.tile_pool(name="const", bufs=1))
        ident = const.tile([128, 128], F32)
        make_identity(nc, ident)
        qsc_sb = const.tile([128, 1], F32)
        nc.sync.dma_start(out=qsc_sb, in_=qsc_d)
        pw_sb = const.tile([64, 128], F32)
        nc.sync.dma_start(out=pw_sb, in_=pw_d)
        w1_sb = const.tile([128, 384], F32)
        nc.sync.dma_start(out=w1_sb, in_=w1_d)
        w1r = const.tile([128, 384], F32R)
        nc.vector.tensor_copy(w1r, w1_sb)

        # long-lived activations
        longp = ctx.enter_context(tc.tile_pool(name="long", bufs=1))
        qk_sb = longp.tile([128, HW], F32)   # part: q ch 0-63 | k ch 0-63
        v_sb = longp.tile([128, HW // 2], F32)  # part c+64*(h//64), free (h%64)*W+w

        with contextlib.ExitStack() as cctx:
            # ---------------- conv phase pools ----------------
            convc = cctx.enter_context(tc.tile_pool(name="convc", bufs=1))
            w2r = convc.tile([128, 27, 192], F32R)
            with tc.tile_pool(name="w2tmp", bufs=1) as w2tmp:
                w2_sb = w2tmp.tile([128, 27, 192], F32)
                nc.sync.dma_start(out=w2_sb,
                                  in_=w2_d.rearrange("t p o -> p t o"))
                nc.vector.tensor_copy(w2r, w2_sb)

            xpool = cctx.enter_context(tc.tile_pool(name="xpool", bufs=2))
            xrpool = cctx.enter_context(tc.tile_pool(name="xrpool", bufs=2))
            p1pool = cctx.enter_context(
                tc.tile_pool(name="p1pool", bufs=2, space="PSUM"))
            pqkpool = cctx.enter_context(
                tc.tile_pool(name="pqkpool", bufs=1, space="PSUM"))
            pvpool = cctx.enter_context(
                tc.tile_pool(name="pvpool", bufs=1, space="PSUM"))
            sqpool = cctx.enter_context(tc.tile_pool(name="sqpool", bufs=1))
            nrmpool = cctx.enter_context(tc.tile_pool(name="nrmpool", bufs=4))

            # t buffers: manual 3-deep ring, padded (W+2) with zero border
            # cols; conv1 writes cols 1..128 only, so borders stay zero.
            # (memset can't write f32r, so borders are zeroed via a rounding
            # copy from a zeroed f32 column.)
            zcol = convc.tile([128, TROWS, 1], F32)
            nc.vector.memset(zcol, 0.0)
            tbufs = []
            for d in range(3):
                row = []
                for it in range(3):
                    t = convc.tile([128, TROWS, W + 2], F32R,
                                   name=f"tbuf{d}_{it}")
                    nc.vector.tensor_copy(t[:, :, 0:1], zcol)
                    nc.vector.tensor_copy(t[:, :, W + 1:W + 2], zcol)
                    row.append(t)
                tbufs.append(row)

            ssq = convc.tile([128, 128], F32)  # sum(q^2 / k^2) per (c, h)

            def conv1_block(b):
                r0 = b * BLKR
                lo = r0 - 1  # global x row of t-slot 0
                x_tmp = xpool.tile([128, TROWS, W], F32)
                vs = max(0, -lo)
                ve = TROWS - (1 if b == NBLK - 1 else 0)
                nc.sync.dma_start(
                    out=x_tmp[:, vs:ve, :],
                    in_=x_d[:, (lo + vs) * W:(lo + ve) * W].rearrange(
                        "p (r w) -> p r w", w=W))
                if b == 0:
                    nc.vector.memset(x_tmp[:, 0:1, :], 0.0)
                if b == NBLK - 1:
                    nc.vector.memset(x_tmp[:, TROWS - 1:TROWS, :], 0.0)
                xr = xrpool.tile([128, TROWS, W], F32R)
                nc.vector.tensor_copy(xr, x_tmp)
                for it in range(3):
                    p1 = p1pool.tile([128, TROWS * W], F32, name="p1")
                    for s, (ls, nr) in enumerate(((0, 4), (4, 4), (8, 2))):
                        nc.tensor.matmul(
                            p1[:, s * 512:s * 512 + nr * W],
                            w1r[:, it * 128:(it + 1) * 128],
                            xr[:, ls:ls + nr, :],
                            start=True, stop=True)
                    nc.vector.tensor_copy(
                        tbufs[b % 3][it][:, :, 1:W + 1],
                        p1.rearrange("p (r w) -> p r w", w=W))

            def conv2_block(b):
                r0 = b * BLKR
                tb = tbufs[b % 3]
                for sub in range(2):
                    y0 = r0 + 4 * sub
                    psqk = pqkpool.tile([128, 4, W], F32, name="psqk")
                    psv = pvpool.tile([64, 4, W], F32, name="psv")
                    idx = 0
                    for it in range(3):
                        for dy in range(3):
                            for dx in range(3):
                                t = it * 9 + dy * 3 + dx
                                rhs = tb[it][:, 4 * sub + dy:4 * sub + dy + 4,
                                             dx:dx + W]
                                nc.tensor.matmul(
                                    psqk, w2r[:, t, 0:128], rhs,
                                    start=(idx == 0), stop=(idx == 26))
                                idx += 1
                    idx = 0
                    for it in range(3):
                        for dy in range(3):
                            for dx in range(3):
                                t = it * 9 + dy * 3 + dx
                                rhs = tb[it][:, 4 * sub + dy:4 * sub + dy + 4,
                                             dx:dx + W]
                                nc.tensor.matmul(
                                    psv, w2r[:, t, 128:192], rhs,
                                    start=(idx == 0), stop=(idx == 26))
                                idx += 1
                    nc.scalar.copy(
                        out=qk_sb[:, y0 * W:y0 * W + 512],
                        in_=psqk.rearrange("p r w -> p (r w)"))
                    hh = y0 // 64
                    nc.scalar.copy(
                        out=v_sb[64 * hh:64 * hh + 64,
                                 (y0 % 64) * W:(y0 % 64) * W + 512],
                        in_=psv.rearrange("p r w -> p (r w)"))

            def phasec_chunk(c):
                # l2 norm of q,k rows (over w) for h rows 32c..32c+31,
                # q rows additionally scaled by `scale`.
                view = qk_sb[:, c * 32 * W:(c + 1) * 32 * W].rearrange(
                    "p (r w) -> p r w", w=W)
                for q in range(2):
                    sq = sqpool.tile([128, 16, W], F32, name="sq")
                    v16 = view[:, q * 16:(q + 1) * 16, :]
                    nc.vector.tensor_mul(sq, v16, v16)
                    nc.vector.reduce_sum(
                        out=ssq[:, c * 32 + q * 16:c * 32 + q * 16 + 16],
                        in_=sq, axis=AX.X)
                nrm = nrmpool.tile([128, 32], F32, name="nrm")
                nc.scalar.activation(nrm, ssq[:, c * 32:(c + 1) * 32], AF.Sqrt)
                rn = nrmpool.tile([128, 32], F32, name="rn")
                nc.vector.reciprocal(rn, nrm)
                nc.vector.tensor_scalar_mul(rn, rn, qsc_sb)
                nc.vector.tensor_mul(
                    view, view, rn.unsqueeze(2).broadcast_to((128, 32, W)))

            conv1_block(0)
            conv1_block(1)
            for b in range(NBLK):
                if b + 2 < NBLK:
                    conv1_block(b + 2)
                conv2_block(b)
                if b % 4 == 3:
                    phasec_chunk(b // 4)

        # ---------------- attention phase ----------------
        with contextlib.ExitStack() as actx:
            o2pool = actx.enter_context(tc.tile_pool(name="o2pool", bufs=1))
            o2_sb = o2pool.tile([64, HW], F32)
            attn_inner = actx.enter_context(contextlib.ExitStack())
            ainp = attn_inner.enter_context(tc.tile_pool(name="ainp", bufs=6))
            asb = attn_inner.enter_context(tc.tile_pool(name="asb", bufs=8))
            asmall = attn_inner.enter_context(
                tc.tile_pool(name="asmall", bufs=8))
            tpsp = attn_inner.enter_context(
                tc.tile_pool(name="tpsp", bufs=2, space="PSUM"))
            spsp = attn_inner.enter_context(
                tc.tile_pool(name="spsp", bufs=1, space="PSUM"))
            opsp = attn_inner.enter_context(
                tc.tile_pool(name="opsp", bufs=1, space="PSUM"))

            for c in range(64):
                qc = ainp.tile([128, 128], F32, name="qc")
                nc.sync.dma_start(out=qc, in_=qk_sb[c:c + 1, :])
                kc = ainp.tile([128, 128], F32, name="kc")
                nc.sync.dma_start(out=kc, in_=qk_sb[64 + c:65 + c, :])
                vc = ainp.tile([128, 128], F32, name="vc")
                for hh in range(2):
                    nc.sync.dma_start(
                        out=vc[64 * hh:64 * hh + 64, :],
                        in_=v_sb[64 * hh + c:64 * hh + c + 1, :])

                qt_ps = tpsp.tile([128, 128], F32, name="qt_ps")
                nc.tensor.transpose(qt_ps, qc, ident)
                qt = asb.tile([128, 128], F32, name="qt")
                nc.vector.tensor_copy(qt, qt_ps)
                kt_ps = tpsp.tile([128, 128], F32, name="kt_ps")
                nc.tensor.transpose(kt_ps, kc, ident)
                kt = asb.tile([128, 128], F32, name="kt")
                nc.scalar.copy(kt, kt_ps)

                s_ps = spsp.tile([128, 128], F32, name="s_ps")
                nc.tensor.matmul(s_ps, qt, kt, start=True, stop=True)
                negm = asmall.tile([128, 1], F32, name="negm")
                nc.vector.reduce_max(negm, s_ps, axis=AX.X, negate=True)
                p_sb = asb.tile([128, 128], F32, name="p_sb")
                sume = asmall.tile([128, 1], F32, name="sume")
                nc.scalar.activation(p_sb, s_ps, AF.Exp, bias=negm,
                                     scale=1.0, accum_out=sume)
                rs = asmall.tile([128, 1], F32, name="rs")
                nc.vector.reciprocal(rs, sume)

                pt_ps = tpsp.tile([128, 128], F32, name="pt_ps")
                nc.tensor.transpose(pt_ps, p_sb, ident)
                pt = asb.tile([128, 128], F32, name="pt")
                nc.vector.tensor_copy(pt, pt_ps)
                o_ps = opsp.tile([128, 128], F32, name="o_ps")
                nc.tensor.matmul(o_ps, pt, vc, start=True, stop=True)
                o_sb = asb.tile([128, 128], F32, name="o_sb")
                nc.vector.tensor_scalar_mul(o_sb, o_ps, rs)
                nc.sync.dma_start(out=o2_sb[c:c + 1, :], in_=o_sb)

            # ---------------- proj ----------------
            attn_inner.close()
            ypsp = actx.enter_context(
                tc.tile_pool(name="ypsp", bufs=4, space="PSUM"))
            ysbp = actx.enter_context(tc.tile_pool(name="ysbp", bufs=3))
            for s in range(HW // 512):
                y_ps = ypsp.tile([128, 512], F32, name="y_ps")
                nc.tensor.matmul(y_ps, pw_sb, o2_sb[:, s * 512:(s + 1) * 512],
                                 start=True, stop=True)
                y_sb = ysbp.tile([128, 512], F32, name="y_sb")
                nc.vector.tensor_copy(y_sb, y_ps)
                nc.sync.dma_start(out=y_d[:, s * 512:(s + 1) * 512], in_=y_sb)


def build_nc():
    nc = bacc.Bacc("TRN2", target_bir_lowering=False, debug=False,
                   num_devices=NCORES)
    x_d = nc.dram_tensor("x", [C, HW], F32, kind="ExternalInput").ap()
    w1_d = nc.dram_tensor("w1t", [128, 384], F32, kind="ExternalInput").ap()
    w2_d = nc.dram_tensor("w2t", [27, 128, 192], F32,
                          kind="ExternalInput").ap()
    pw_d = nc.dram_tensor("pwt", [64, 128], F32, kind="ExternalInput").ap()
    qsc_d = nc.dram_tensor("qsc", [128, 1], F32, kind="ExternalInput").ap()
    y_d = nc.dram_tensor("y", [C, HW], F32, kind="ExternalOutput").ap()

    with tile.TileContext(nc) as tc:
        _trace_kernel(tc, x_d, w1_d, w2_d, pw_d, qsc_d, y_d)
    nc.compile()
    return nc


_CACHE_LOCK = threading.Lock()
_CACHED_NC = None


def get_nc():
    global _CACHED_NC
    with _CACHE_LOCK:
        if _CACHED_NC is None:
            _CACHED_NC = build_nc()
        return _CACHED_NC


def prep_core_inputs(x, qkv1_w, qkv2_w, proj_w, scale, core):
    b, half = divmod(core, 2)
    ch0 = 64 * half
    sel = np.concatenate([np.arange(ch0, ch0 + 64),
                          128 + np.arange(ch0, ch0 + 64),
                          256 + np.arange(ch0, ch0 + 64)])
    xc = np.ascontiguousarray(x[b].reshape(C, HW), dtype=np.float32)
    w1t = np.ascontiguousarray(qkv1_w[:, :, 0, 0].T, dtype=np.float32)
    w2 = qkv2_w[sel]  # (192, 384, 3, 3)
    w2t = np.ascontiguousarray(
        w2.transpose(1, 2, 3, 0).reshape(3, 128, 3, 3, 192)
        .transpose(0, 2, 3, 1, 4).reshape(27, 128, 192), dtype=np.float32)
    pwt = np.ascontiguousarray(proj_w[:, ch0:ch0 + 64, 0, 0].T,
                               dtype=np.float32)
    qsc = np.concatenate([scale[ch0:ch0 + 64, 0, 0],
                          np.ones(64, np.float32)]).reshape(128, 1)
    return {"x": xc, "w1t": w1t, "w2t": w2t, "pwt": pwt,
            "qsc": np.ascontiguousarray(qsc, dtype=np.float32)}


def run_sharded(inputs, trace=False):
    """Run on 8 cores; returns (y_full, BassKernelResults)."""
    x = np.asarray(inputs["x"], dtype=np.float32)
    qkv1_w = np.asarray(inputs["qkv1_w"], dtype=np.float32)
    qkv2_w = np.asarray(inputs["qkv2_w"], dtype=np.float32)
    proj_w = np.asarray(inputs["proj_w"], dtype=np.float32)
    scale = np.asarray(inputs["scale"], dtype=np.float32)

    nc = get_nc()
    in_maps = [prep_core_inputs(x, qkv1_w, qkv2_w, proj_w, scale, c)
               for c in range(NCORES)]
    res = run_bass_kernel_spmd(nc, in_maps, list(range(NCORES)), trace=trace)
    y = np.empty((B, C, H, W), np.float32)
    for b in range(B):
        y[b] = (res.results[2 * b]["y"]
                + res.results[2 * b + 1]["y"]).reshape(C, H, W)
    return y, res


def kernel(**inputs) -> np.ndarray:
    y, _ = run_sharded(inputs, trace=False)
    return y
